# revision 1
# baseline (speedup 1.0000x reference)
import numpy as np
import jax
import jax.numpy as jnp

# Problem constants (hardcoded per contract: kernel.py is self-contained).
N_DST, L_MAX, D, VOCAB, N_CLS = 50000, 16, 256, 50000, 104
M = 8  # cores; data-parallel over destination nodes, params replicated.


def _compute(msg_tokens, degrees, emb, W_ih, W_hh, b_ih, b_hh,
             ln_gamma, ln_beta, fc_w, fc_b):
    msgs = emb[msg_tokens]                                   # [n, L, D]
    pos = jnp.arange(L_MAX)
    hmask = (pos[None, :] < (degrees - 1)[:, None]).astype(msgs.dtype)
    h0 = jnp.einsum('nld,nl->nd', msgs, hmask)               # [n, D]
    last = jnp.take_along_axis(
        msgs, (degrees - 1)[:, None, None], axis=1)[:, 0, :]
    gi = last @ W_ih.T + b_ih
    gh = h0 @ W_hh.T + b_hh
    i_r, i_z, i_n = jnp.split(gi, 3, axis=-1)
    h_r, h_z, h_n = jnp.split(gh, 3, axis=-1)
    r = jax.nn.sigmoid(i_r + h_r)
    z = jax.nn.sigmoid(i_z + h_z)
    n = jnp.tanh(i_n + r * h_n)
    h1 = (1.0 - z) * n + z * h0
    mu = jnp.mean(h1, axis=-1, keepdims=True)
    var = jnp.var(h1, axis=-1, keepdims=True)
    ln = (h1 - mu) * jax.lax.rsqrt(var + 1e-5) * ln_gamma + ln_beta
    ft = jnp.where((degrees == 1)[:, None], last, ln)
    return ft @ fc_w.T + fc_b                                # [n, N_CLS]


def kernel(msg_tokens, degrees, emb, W_ih, W_hh, b_ih, b_hh,
           ln_gamma, ln_beta, fc_w, fc_b):
    msg_tokens = np.asarray(msg_tokens)
    degrees = np.asarray(degrees)
    n_dst = msg_tokens.shape[0]
    try:
        devs = jax.devices()
        if len(devs) >= M and n_dst % M == 0:
            per = n_dst // M
            pc = jax.pmap(
                _compute,
                in_axes=(0, 0, None, None, None, None, None, None, None,
                         None, None),
                devices=devs[:M])
            out = pc(msg_tokens.reshape(M, per, L_MAX),
                     degrees.reshape(M, per),
                     emb, W_ih, W_hh, b_ih, b_hh,
                     ln_gamma, ln_beta, fc_w, fc_b)
            return np.asarray(out).reshape(n_dst, N_CLS)
    except Exception:
        pass
    cpu = jax.devices('cpu')[0]
    with jax.default_device(cpu):
        out = jax.jit(_compute)(msg_tokens, degrees, emb, W_ih, W_hh,
                                b_ih, b_hh, ln_gamma, ln_beta, fc_w, fc_b)
    return np.asarray(out)

